# revision 11
# baseline (speedup 1.0000x reference)
"""Trainium2 Bass kernel for nn_Attention_82343112999676.

Multi-head attention (B=4, S=1024, DIM=4096, H=32, HD=128) with LoRA on the
q/k/v/o projections and a tanh-gated adapter cross-attention path.

Distribution: tensor-parallel over heads (4 heads / core on 8 NeuronCores).
Each core projects q/k/v for its heads (weights sharded column-wise, LoRA
replicated), runs RoPE + both attention paths locally, then an AllToAll
reshards the attention output from head-sharded to token-sharded so the
output projection (wo row-parallel + LoRA) runs without any AllReduce.

Layout strategy: activations are kept transposed ([feature, token]) so every
matmul contraction lands on the SBUF partition axis with zero on-device
transposes of x or weights (the host feeds pre-transposed bf16 weights).
Scores are computed transposed ([tk, tq]); softmax sums come from a
ones-column matmul and the normalization happens after the attention output
returns to natural [token, feature] layout, where per-partition reciprocals
are cheap.

RoPE trick: the host permutes the q/k weight rows (evens-then-odds per head)
so the complex-pair rotation becomes contiguous half-partition operations.
Attention is invariant to this permutation since q and k are permuted alike.
"""

import math
from contextlib import ExitStack

import numpy as np
import ml_dtypes

bf16 = ml_dtypes.bfloat16

B, S, DIM, H, HD, R, AL = 4, 1024, 4096, 32, 128, 16, 10
NCORES = 8
HL = H // NCORES          # heads per core = 4
QO = HL * HD              # per-core projected features = 512
NI = DIM // 128           # contraction tiles = 32
NT2 = S // 512            # tq 512-blocks per batch = 2
NJ = S // 128             # tk 128-tiles per batch = 8
SCALE = 1.0 / math.sqrt(HD)

_CACHE = {}


# --------------------------------------------------------------------------
# mask classification
# --------------------------------------------------------------------------
def _classify_mask(mask):
    """Block classification of exp(mask) over (tq 512-blocks, tk 128-tiles)."""
    em = np.exp(np.asarray(mask, np.float64)[0, 0]).astype(np.float32)  # [S, S]
    cls = {}
    gen = []
    for T in range(NT2):
        for j in range(NJ):
            blk = em[T * 512:(T + 1) * 512, j * 128:(j + 1) * 128]  # [tq, tk]
            if np.all(blk == 1.0):
                cls[(T, j)] = ("one", -1)
            elif np.all(blk == 0.0):
                cls[(T, j)] = ("zero", -1)
            else:
                blkT = np.ascontiguousarray(blk.T).astype(bf16)  # [tk, tq]
                for gi, gb in enumerate(gen):
                    if gb.tobytes() == blkT.tobytes():
                        cls[(T, j)] = ("gen", gi)
                        break
                else:
                    cls[(T, j)] = ("gen", len(gen))
                    gen.append(blkT)
    subzero = {}
    for g in range(NJ):
        for j in range(NJ):
            blk = em[g * 128:(g + 1) * 128, j * 128:(j + 1) * 128]
            subzero[(g, j)] = bool(np.all(blk == 0.0))
    key_parts = [cls[(T, j)][0] for T in range(NT2) for j in range(NJ)]
    gen_bytes = b"".join(g.tobytes() for g in gen)
    key = (tuple(key_parts), hash(gen_bytes))
    meta = {"cls": cls, "gen": gen, "subzero": subzero, "n_gen": len(gen)}
    return key, meta


# --------------------------------------------------------------------------
# device program
# --------------------------------------------------------------------------
_DEBUG = False


def _build(meta):
    import concourse.tile as tile
    from concourse import bacc, mybir
    from concourse.masks import make_identity

    dt = mybir.dt
    f32, bfl = dt.float32, dt.bfloat16
    AF = mybir.ActivationFunctionType
    ALU = mybir.AluOpType

    cls = meta["cls"]
    subzero = meta["subzero"]
    n_gen = meta["n_gen"]
    # per tq-128 tile g: live tk-tiles (for psum start/stop flags)
    live_j = {g: [j for j in range(NJ) if not subzero[(g, j)]] for g in range(NJ)}

    nc = bacc.Bacc("TRN2", target_bir_lowering=False, debug=False,
                   num_devices=NCORES)

    def din(name, shape, d=bfl):
        return nc.dram_tensor(name, shape, d, kind="ExternalInput").ap()

    xT = din("xT", [DIM, B * S])
    wqT = din("wqT", [DIM, QO])
    wkT = din("wkT", [DIM, QO])
    wvT = din("wvT", [DIM, QO])
    woT = din("woT", [DIM, DIM])
    l1a = din("l1a", [DIM, 96])
    l2ad = din("l2all", [96, QO])
    l1od = din("l1o", [DIM, R])
    l2od = din("l2o", [R, DIM])
    adT = din("adT", [DIM, B * 32])
    admd = din("admcol", [32, 1])
    c2d = din("c2", [HD, S])
    s2d = din("s2", [HD, S])
    tgd = din("tanhg", [128, HL], f32)
    embd = din("embl", [n_gen, 128, 512]) if n_gen else None
    out = nc.dram_tensor("out", [B, 128, DIM], f32, kind="ExternalOutput").ap()
    dbg_in = nc.dram_tensor("dbg_in", [B, NCORES, QO, 128], bfl,
                            kind="ExternalOutput").ap() if _DEBUG else None
    dbg_out = nc.dram_tensor("dbg_out", [B, NCORES, QO, 128], bfl,
                             kind="ExternalOutput").ap() if _DEBUG else None
    a2a_in = nc.dram_tensor("a2a_in", [B, NCORES, QO, 128], bfl).ap()
    a2a_out = nc.dram_tensor("a2a_out", [B, NCORES, QO, 128], bfl).ap()

    with tile.TileContext(nc) as tc, ExitStack() as ctx:
        pool = lambda name, bufs, **kw: ctx.enter_context(
            tc.tile_pool(name=name, bufs=bufs, **kw))
        cst = pool("cst", 1)
        qk = pool("qk", 2)
        vpool = pool("vpool", 2)
        xp = pool("xp", 4)
        wvp = pool("wvp", 3)
        sp = pool("sp", 3)        # pt tiles
        rp = pool("rp", 2)        # rope temps
        tpp = pool("tpp", 2)      # lora-down staging
        mp = pool("mp", 3)        # merge temps
        shp = pool("shp", 3)      # att/ship tiles
        wop = pool("wop", 3)      # woT stream
        atp = pool("atp", 6)      # attT stream
        osp = pool("osp", 2)      # out staging
        ps_big = pool("ps_big", 4, space="PSUM")
        ps_sum = pool("ps_sum", 2, space="PSUM")
        ps_tmp = pool("ps_tmp", 2, space="PSUM")

        # ---------------- constants / weights preload ----------------
        wq_sb = cst.tile([128, NI, QO], bfl, tag="wq")
        nc.sync.dma_start(wq_sb[:], wqT[:].rearrange("(a p) f -> p a f", p=128))
        wk_sb = cst.tile([128, NI, QO], bfl, tag="wk")
        nc.sync.dma_start(wk_sb[:], wkT[:].rearrange("(a p) f -> p a f", p=128))
        l1_sb = cst.tile([128, NI, 96], bfl, tag="l1")
        nc.sync.dma_start(l1_sb[:], l1a[:].rearrange("(a p) f -> p a f", p=128))
        l2a_sb = cst.tile([96, QO], bfl, tag="l2a")
        nc.sync.dma_start(l2a_sb[:], l2ad[:])
        admcol = cst.tile([32, 1], bfl, tag="admcol")
        nc.sync.dma_start(admcol[:], admd[:])
        l1o_sb = cst.tile([128, NI, R], bfl, tag="l1o")
        nc.sync.dma_start(l1o_sb[:], l1od[:].rearrange("(a p) f -> p a f", p=128))
        ad_sb = cst.tile([128, NI, B * 32], bfl, tag="ad")
        nc.sync.dma_start(ad_sb[:], adT[:].rearrange("(a p) f -> p a f", p=128))
        c2_sb = cst.tile([128, S], bfl, tag="c2")
        nc.sync.dma_start(c2_sb[:], c2d[:])
        s2_sb = cst.tile([128, S], bfl, tag="s2")
        nc.sync.dma_start(s2_sb[:], s2d[:])
        tg_sb = cst.tile([128, HL], f32, tag="tg")
        nc.sync.dma_start(tg_sb[:], tgd[:])
        if n_gen:
            em_sb = cst.tile([128, n_gen, 512], bfl, tag="em")
            nc.sync.dma_start(em_sb[:], embd[:].rearrange("g p f -> p g f"))
        ident = cst.tile([128, 128], bfl, tag="ident")
        make_identity(nc, ident[:])
        onecol = cst.tile([128, 1], bfl, tag="onecol")
        nc.vector.memset(onecol[:], 1.0)

        # ---------------- adapter k/v (base weights, no rope/lora) --------
        akn_ps = ps_big.tile([B * 32, QO], f32, tag="big")
        avn_ps = ps_big.tile([B * 32, QO], f32, tag="big")
        for i in range(NI):
            wvt = wvp.tile([128, QO], bfl, tag="wv")
            nc.sync.dma_start(wvt[:], wvT[i * 128:(i + 1) * 128, :])
            nc.tensor.matmul(akn_ps[:], ad_sb[:, i, :], wk_sb[:, i, :],
                             start=(i == 0), stop=(i == NI - 1))
            nc.tensor.matmul(avn_ps[:], ad_sb[:, i, :], wvt[:],
                             start=(i == 0), stop=(i == NI - 1))
        akn_sb = cst.tile([B * 32, QO], bfl, tag="akn")
        nc.scalar.copy(akn_sb[:], akn_ps[:])
        # av in [32, B, HL, HD] so each batch slice starts at partition 0
        av_sb = cst.tile([32, B, HL, HD], bfl, tag="av")
        for b2 in range(B):
            nc.vector.tensor_copy(
                av_sb[:, b2].rearrange("p h d -> p (h d)"),
                avn_ps[b2 * 32:(b2 + 1) * 32, :])
        akT_sb = cst.tile([128, HL, B * 32], bfl, tag="akT")
        for hh in range(HL):
            trp = ps_tmp.tile([128, B * 32], bfl, tag="tmp")
            nc.tensor.transpose(trp[:], akn_sb[:, hh * HD:(hh + 1) * HD],
                                ident[:])
            nc.scalar.copy(akT_sb[:, hh, :], trp[:])

        # ---------------- per-batch: projection + attention ----------------
        for b in range(B):
            qT_sb = qk.tile([128, HL, S], bfl, tag="qT")
            kT_sb = qk.tile([128, HL, S], bfl, tag="kT")
            v_sb = vpool.tile([128, NJ, QO], bfl, tag="v")

            for t2 in range(NT2):
                ts = t2 * 512
                tg0 = b * S + ts
                xtc = []
                for cchunk in range(4):
                    xt = xp.tile([128, 8, 512], bfl, tag="xt")
                    src = xT[cchunk * 1024:(cchunk + 1) * 1024, tg0:tg0 + 512]
                    nc.sync.dma_start(xt[:], src.rearrange("(a p) t -> p a t", p=128))
                    xtc.append(xt)

                def xtile(i):
                    return xtc[i // 8][:, i % 8, :]

                def rope_store(ps, dst, ts_):
                    cs = c2_sb[:, ts_:ts_ + 512]
                    sn = s2_sb[:, ts_:ts_ + 512]
                    qb = rp.tile([128, 512], bfl, tag="ropeq")
                    nc.scalar.copy(qb[:], ps[:])
                    sw = rp.tile([128, 512], bfl, tag="ropesw")
                    nc.vector.tensor_copy(sw[0:64, :], qb[64:128, :])
                    nc.vector.tensor_copy(sw[64:128, :], qb[0:64, :])
                    nc.vector.tensor_mul(sw[:], sw[:], sn)
                    nc.vector.tensor_mul(qb[:], qb[:], cs)
                    nc.vector.tensor_add(dst, qb[:], sw[:])

                # ---- pass 1: q heads + lora down-projection ----
                tmp_ps = ps_tmp.tile([96, 512], f32, tag="tmp")
                qps = [ps_big.tile([128, 512], f32, tag="big", name=f"qps{_}") for _ in range(HL)]
                for i in range(NI):
                    for hh in range(HL):
                        nc.tensor.matmul(qps[hh][:],
                                         wq_sb[:, i, hh * HD:(hh + 1) * HD],
                                         xtile(i), start=(i == 0), stop=False)
                    nc.tensor.matmul(tmp_ps[:], l1_sb[:, i, :], xtile(i),
                                     start=(i == 0), stop=(i == NI - 1))
                tmp_sb = tpp.tile([96, 512], bfl, tag="tmps")
                nc.scalar.copy(tmp_sb[:], tmp_ps[:])
                for hh in range(HL):
                    nc.tensor.matmul(qps[hh][:], l2a_sb[0:R, hh * HD:(hh + 1) * HD],
                                     tmp_sb[0:R, :], start=False, stop=True)
                    rope_store(qps[hh], qT_sb[:, hh, ts:ts + 512], ts)

                # ---- pass 2: k heads ----
                kps = [ps_big.tile([128, 512], f32, tag="big", name=f"kps{_}") for _ in range(HL)]
                for i in range(NI):
                    for hh in range(HL):
                        nc.tensor.matmul(kps[hh][:],
                                         wk_sb[:, i, hh * HD:(hh + 1) * HD],
                                         xtile(i), start=(i == 0), stop=False)
                for hh in range(HL):
                    nc.tensor.matmul(kps[hh][:], l2a_sb[32:32 + R, hh * HD:(hh + 1) * HD],
                                     tmp_sb[32:32 + R, :], start=False, stop=True)
                    rope_store(kps[hh], kT_sb[:, hh, ts:ts + 512], ts)

                # ---- pass 3: v natural [token, feature] ----
                vps = [ps_big.tile([128, 512], f32, tag="big", name=f"vps{_}") for _ in range(4)]
                for i in range(NI):
                    wvt = wvp.tile([128, QO], bfl, tag="wv")
                    nc.sync.dma_start(wvt[:], wvT[i * 128:(i + 1) * 128, :])
                    for sub in range(4):
                        nc.tensor.matmul(vps[sub][:],
                                         xtile(i)[:, sub * 128:(sub + 1) * 128],
                                         wvt[:], start=(i == 0), stop=False)
                for sub in range(4):
                    nc.tensor.matmul(vps[sub][:],
                                     tmp_sb[64:64 + R, sub * 128:(sub + 1) * 128],
                                     l2a_sb[64:64 + R, :], start=False, stop=True)
                    nc.scalar.copy(v_sb[:, t2 * 4 + sub, :], vps[sub][:])

            # ---- attention for batch b ----
            for hh in range(HL):
                for T in range(NT2):
                    out_data = ps_big.tile([128, 512], f32, tag="big")
                    ad_data = ps_big.tile([128, 512], f32, tag="big")
                    sums = ps_sum.tile([128, 8], f32, tag="sums")
                    # PSUM start=True clears has_written for the WHOLE bank, so
                    # each psum tile gets exactly one start=True (its first mm);
                    # later mms overwrite-fresh / accumulate per element.
                    started = {"out": False, "ad": False, "sums": False}

                    def gmm(which, out_ap, lhsT, rhs, stop):
                        nc.tensor.matmul(out_ap, lhsT, rhs,
                                         start=not started[which], stop=stop)
                        started[which] = True

                    for j in range(NJ):
                        kind, gidx = cls[(T, j)]
                        if kind == "zero":
                            continue
                        sc = ps_big.tile([128, 512], f32, tag="big")
                        nc.tensor.matmul(sc[:], kT_sb[:, hh, j * 128:(j + 1) * 128],
                                         qT_sb[:, hh, T * 512:(T + 1) * 512],
                                         start=True, stop=True)
                        pt = sp.tile([128, 512], bfl, tag="pt")
                        nc.scalar.activation(pt[:], sc[:], AF.Exp, scale=SCALE)
                        if kind == "gen":
                            nc.vector.tensor_mul(pt[:], pt[:], em_sb[:, gidx, :])
                        for s_ in range(4):
                            g = T * 4 + s_
                            if subzero[(g, j)]:
                                continue
                            stp = (j == live_j[g][-1])
                            gmm("out", out_data[:, s_ * 128:(s_ + 1) * 128],
                                pt[:, s_ * 128:(s_ + 1) * 128],
                                v_sb[:, j, hh * HD:(hh + 1) * HD], stp)
                            gmm("sums", sums[:, s_:s_ + 1],
                                pt[:, s_ * 128:(s_ + 1) * 128], onecol[:], stp)
                    # adapter cross-attention for (hh, T)
                    adsc = ps_big.tile([32, 512], f32, tag="big")
                    nc.tensor.matmul(adsc[:], akT_sb[:, hh, b * 32:(b + 1) * 32],
                                     qT_sb[:, hh, T * 512:(T + 1) * 512],
                                     start=True, stop=True)
                    pad = sp.tile([32, 512], bfl, tag="pad")
                    nc.scalar.activation(pad[:], adsc[:], AF.Exp, scale=SCALE)
                    for s_ in range(4):
                        gmm("ad", ad_data[:, s_ * 128:(s_ + 1) * 128],
                            pad[:, s_ * 128:(s_ + 1) * 128],
                            av_sb[:, b, hh, :], True)
                        gmm("sums", sums[:, 4 + s_:5 + s_],
                            pad[:, s_ * 128:(s_ + 1) * 128], admcol[:], True)
                    # normalize + merge + transpose + ship
                    rcpt = mp.tile([128, 16], f32, tag="rcp")
                    for s_ in range(4):
                        g = T * 4 + s_
                        att = shp.tile([128, 128], bfl, tag="att")
                        if not live_j[g]:
                            nc.vector.memset(att[:], 0.0)
                        else:
                            rc = rcpt[:, s_:s_ + 1]
                            rca = rcpt[:, 8 + s_:9 + s_]
                            nc.vector.reciprocal(rc, sums[:, s_:s_ + 1])
                            nc.vector.reciprocal(rca, sums[:, 4 + s_:5 + s_])
                            rca2 = rcpt[:, 12 + s_:13 + s_]
                            nc.vector.tensor_mul(rca2, rca, tg_sb[:, hh:hh + 1])
                            tmpm = mp.tile([128, 128], f32, tag="mtmp")
                            nc.vector.tensor_scalar_mul(
                                tmpm[:], ad_data[:, s_ * 128:(s_ + 1) * 128], rca2)
                            nc.vector.scalar_tensor_tensor(
                                out=att[:],
                                in0=out_data[:, s_ * 128:(s_ + 1) * 128],
                                scalar=rc, in1=tmpm[:],
                                op0=ALU.mult, op1=ALU.add)
                        trp = ps_tmp.tile([128, 128], bfl, tag="tmp")
                        nc.tensor.transpose(trp[:], att[:], ident[:])
                        attt = shp.tile([128, 128], bfl, tag="attt")
                        nc.scalar.copy(attt[:], trp[:])
                        nc.sync.dma_start(
                            a2a_in[b, g, hh * HD:(hh + 1) * HD, :], attt[:])
            nc.gpsimd.collective_compute(
                "AllToAll", mybir.AluOpType.bypass,
                ins=[a2a_in[b]], outs=[a2a_out[b]],
                replica_groups=[list(range(NCORES))],
            )

        # ---------------- wo projection (token-parallel) ----------------
        # All four AllToAlls must have landed before their outputs are read.
        tc.strict_bb_all_engine_barrier()
        if _DEBUG:
            nc.sync.dma_start(dbg_in[:], a2a_in[:])
            nc.sync.dma_start(dbg_out[:], a2a_out[:])
        # attT af-tile a = (src core a//HL, local head a%HL); token group = core id.
        tmpo_sb = cst.tile([R, B, 128], bfl, tag="tmpo")
        for b2 in range(B):
            top = ps_tmp.tile([R, 128], f32, tag="tmp")
            for a in range(NI):
                at = atp.tile([128, 128], bfl, tag="at")
                nc.sync.dma_start(
                    at[:], a2a_out[b2, a // HL, (a % HL) * HD:(a % HL + 1) * HD, :])
                nc.tensor.matmul(top[:], l1o_sb[:, a, :], at[:],
                                 start=(a == 0), stop=(a == NI - 1))
            nc.scalar.copy(tmpo_sb[:, b2, :], top[:])

        for o in range(8):
            l2ot = wop.tile([R, 512], bfl, tag="l2o")
            nc.sync.dma_start(l2ot[:], l2od[:, o * 512:(o + 1) * 512])
            wps = [ps_big.tile([128, 512], f32, tag="big", name=f"wps{_}") for _ in range(B)]
            for a in range(NI):
                wot = wop.tile([128, 512], bfl, tag="wot")
                nc.sync.dma_start(
                    wot[:], woT[a * 128:(a + 1) * 128, o * 512:(o + 1) * 512])
                for b2 in range(B):
                    at = atp.tile([128, 128], bfl, tag="at")
                    nc.sync.dma_start(
                        at[:],
                        a2a_out[b2, a // HL, (a % HL) * HD:(a % HL + 1) * HD, :])
                    nc.tensor.matmul(wps[b2][:], at[:], wot[:],
                                     start=(a == 0), stop=False)
            for b2 in range(B):
                nc.tensor.matmul(wps[b2][:], tmpo_sb[:, b2, :], l2ot[:],
                                 start=False, stop=True)
                ost = osp.tile([128, 512], f32, tag="ost")
                nc.scalar.copy(ost[:], wps[b2][:])
                nc.sync.dma_start(out[b2, :, o * 512:(o + 1) * 512], ost[:])

    nc.compile()
    return nc


# --------------------------------------------------------------------------
# host side
# --------------------------------------------------------------------------
def _prep_inputs(inputs, meta):
    f = lambda k: np.asarray(inputs[k], np.float32)
    x = f("x").reshape(B * S, DIM)
    xT = np.ascontiguousarray(x.astype(bf16).T)                  # [DIM, B*S]
    ad_pad = np.zeros((B, 32, DIM), np.float32)
    ad_pad[:, :AL, :] = f("adapter")
    adT = np.ascontiguousarray(ad_pad.reshape(B * 32, DIM).astype(bf16).T)
    admcol = np.zeros((32, 1), np.float32)
    admcol[:AL] = 1.0
    admcol = admcol.astype(bf16)
    wq, wk, wv, wo = f("wq"), f("wk"), f("wv"), f("wo")
    l2q_, l2k_, l2v_ = f("lora_wq_l2"), f("lora_wk_l2"), f("lora_wv_l2")
    l1a_ = np.zeros((96, DIM), np.float32)
    l1a_[0:R] = f("lora_wq_l1")
    l1a_[32:32 + R] = f("lora_wk_l1")
    l1a_[64:64 + R] = f("lora_wv_l1")
    l1a = np.ascontiguousarray(l1a_.astype(bf16).T)              # [DIM, 96]
    woT = np.ascontiguousarray(wo.astype(bf16).T)                # [DIM, DIM]
    l1o = np.ascontiguousarray(f("lora_wo_l1").astype(bf16).T)   # [DIM, R]
    l2o = np.ascontiguousarray(f("lora_wo_l2").astype(bf16).T)   # [R, DIM]
    cos, sin = f("freqs_cos"), f("freqs_sin")                    # [S, HD/2]
    c2 = np.ascontiguousarray(np.concatenate([cos.T, cos.T], axis=0).astype(bf16))
    s2 = np.ascontiguousarray(np.concatenate([-sin.T, sin.T], axis=0).astype(bf16))
    gate = f("gate").reshape(H)
    perm = np.concatenate([np.arange(0, HD, 2), np.arange(1, HD, 2)])

    gen = meta["gen"]
    embl = np.stack(gen, axis=0) if gen else None

    in_maps = []
    for c in range(NCORES):
        rows_n = c * QO + np.arange(QO)                          # natural rows
        rows_p = np.concatenate(
            [(c * HL + hh) * HD + perm for hh in range(HL)])     # permuted rows
        tanhg = np.broadcast_to(
            np.tanh(gate[c * HL:(c + 1) * HL]).astype(np.float32), (128, HL))
        l2all_ = np.zeros((96, QO), np.float32)
        l2all_[0:R] = l2q_[rows_p].T
        l2all_[32:32 + R] = l2k_[rows_p].T
        l2all_[64:64 + R] = l2v_[rows_n].T
        l2all = np.ascontiguousarray(l2all_.astype(bf16))
        m = {
            "xT": xT,
            "wqT": np.ascontiguousarray(wq[rows_p].astype(bf16).T),
            "wkT": np.ascontiguousarray(wk[rows_p].astype(bf16).T),
            "wvT": np.ascontiguousarray(wv[rows_n].astype(bf16).T),
            "woT": woT,
            "l1a": l1a,
            "l2all": l2all,
            "l1o": l1o,
            "l2o": l2o,
            "adT": adT,
            "c2": c2,
            "s2": s2,
            "tanhg": np.ascontiguousarray(tanhg),
            "admcol": admcol,
        }
        if embl is not None:
            m["embl"] = embl
        in_maps.append(m)
    return in_maps


def _get_program(mask):
    key, meta = _classify_mask(mask)
    if key not in _CACHE:
        _CACHE[key] = (_build(meta), meta)
    return _CACHE[key]


def _run(inputs, trace=False, trace_kwargs=None):
    from concourse.bass_utils import run_bass_kernel_spmd

    nc, meta = _get_program(np.asarray(inputs["mask"], np.float32))
    in_maps = _prep_inputs(inputs, meta)
    res = run_bass_kernel_spmd(nc, in_maps, list(range(NCORES)),
                               trace=trace, **(trace_kwargs or {}))
    final = np.empty((B, S, DIM), np.float32)
    for c in range(NCORES):
        oc = res.results[c]["out"]                               # [B, 128, DIM]
        final[:, c * 128:(c + 1) * 128, :] = oc
    return final, res


def kernel(**inputs) -> np.ndarray:
    out, _ = _run(inputs)
    return out


# revision 13
# speedup vs baseline: 1.1378x; 1.1378x over previous
"""Trainium2 Bass kernel for nn_Attention_82343112999676.

Multi-head attention (B=4, S=1024, DIM=4096, H=32, HD=128) with LoRA on the
q/k/v/o projections and a tanh-gated adapter cross-attention path.

Distribution: tensor-parallel over heads (4 heads / core on 8 NeuronCores).
Each core projects q/k/v for its heads (weights sharded column-wise, LoRA
replicated), runs RoPE + both attention paths locally, then an AllToAll
reshards the attention output from head-sharded to token-sharded so the
output projection (wo row-parallel + LoRA) runs without any AllReduce.

Layout strategy: activations are kept transposed ([feature, token]) so every
matmul contraction lands on the SBUF partition axis with zero on-device
transposes of x or weights (the host feeds pre-transposed bf16 weights).
Scores are computed transposed ([tk, tq]); softmax sums come from a
ones-column matmul and the normalization happens after the attention output
returns to natural [token, feature] layout, where per-partition reciprocals
are cheap.

RoPE trick: the host permutes the q/k weight rows (evens-then-odds per head)
so the complex-pair rotation becomes contiguous half-partition operations.
Attention is invariant to this permutation since q and k are permuted alike.
"""

import math
from contextlib import ExitStack

import numpy as np
import ml_dtypes

bf16 = ml_dtypes.bfloat16

B, S, DIM, H, HD, R, AL = 4, 1024, 4096, 32, 128, 16, 10
NCORES = 8
HL = H // NCORES          # heads per core = 4
QO = HL * HD              # per-core projected features = 512
NI = DIM // 128           # contraction tiles = 32
NT2 = S // 512            # tq 512-blocks per batch = 2
NJ = S // 128             # tk 128-tiles per batch = 8
SCALE = 1.0 / math.sqrt(HD)

_CACHE = {}


# --------------------------------------------------------------------------
# mask classification
# --------------------------------------------------------------------------
def _classify_mask(mask):
    """Block classification of exp(mask) over (tq 512-blocks, tk 128-tiles)."""
    em = np.exp(np.asarray(mask, np.float64)[0, 0]).astype(np.float32)  # [S, S]
    cls = {}
    gen = []
    for T in range(NT2):
        for j in range(NJ):
            blk = em[T * 512:(T + 1) * 512, j * 128:(j + 1) * 128]  # [tq, tk]
            if np.all(blk == 1.0):
                cls[(T, j)] = ("one", -1)
            elif np.all(blk == 0.0):
                cls[(T, j)] = ("zero", -1)
            else:
                blkT = np.ascontiguousarray(blk.T).astype(bf16)  # [tk, tq]
                for gi, gb in enumerate(gen):
                    if gb.tobytes() == blkT.tobytes():
                        cls[(T, j)] = ("gen", gi)
                        break
                else:
                    cls[(T, j)] = ("gen", len(gen))
                    gen.append(blkT)
    subzero = {}
    for g in range(NJ):
        for j in range(NJ):
            blk = em[g * 128:(g + 1) * 128, j * 128:(j + 1) * 128]
            subzero[(g, j)] = bool(np.all(blk == 0.0))
    key_parts = [cls[(T, j)][0] for T in range(NT2) for j in range(NJ)]
    gen_bytes = b"".join(g.tobytes() for g in gen)
    key = (tuple(key_parts), hash(gen_bytes))
    meta = {"cls": cls, "gen": gen, "subzero": subzero, "n_gen": len(gen)}
    return key, meta


# --------------------------------------------------------------------------
# device program
# --------------------------------------------------------------------------
_DEBUG = False


def _build(meta):
    import concourse.tile as tile
    from concourse import bacc, mybir
    from concourse.masks import make_identity

    dt = mybir.dt
    f32, bfl = dt.float32, dt.bfloat16
    AF = mybir.ActivationFunctionType
    ALU = mybir.AluOpType

    cls = meta["cls"]
    subzero = meta["subzero"]
    n_gen = meta["n_gen"]
    # per tq-128 tile g: live tk-tiles (for psum start/stop flags)
    live_j = {g: [j for j in range(NJ) if not subzero[(g, j)]] for g in range(NJ)}

    nc = bacc.Bacc("TRN2", target_bir_lowering=False, debug=False,
                   num_devices=NCORES)

    def din(name, shape, d=bfl):
        return nc.dram_tensor(name, shape, d, kind="ExternalInput").ap()

    xT = din("xT", [DIM, B * S])
    wqT = din("wqT", [DIM, QO])
    wkT = din("wkT", [DIM, QO])
    wvT = din("wvT", [DIM, QO])
    woT = din("woT", [DIM, DIM])
    l1a = din("l1a", [DIM, 96])
    l2ad = din("l2all", [96, QO])
    l1od = din("l1o", [DIM, R])
    l2od = din("l2o", [R, DIM])
    adT = din("adT", [DIM, B * 32])
    admd = din("admcol", [32, 1])
    c2d = din("c2", [HD, S])
    s2d = din("s2", [HD, S])
    tgd = din("tanhg", [128, HL], f32)
    embd = din("embl", [n_gen, 128, 512]) if n_gen else None
    out = nc.dram_tensor("out", [B, 128, DIM], f32, kind="ExternalOutput").ap()
    dbg_in = nc.dram_tensor("dbg_in", [B, NCORES, QO, 128], bfl,
                            kind="ExternalOutput").ap() if _DEBUG else None
    dbg_out = nc.dram_tensor("dbg_out", [B, NCORES, QO, 128], bfl,
                             kind="ExternalOutput").ap() if _DEBUG else None
    a2a_in = nc.dram_tensor("a2a_in", [B, NCORES, QO, 128], bfl).ap()
    a2a_out = nc.dram_tensor("a2a_out", [B, NCORES, QO, 128], bfl).ap()

    with tile.TileContext(nc) as tc, ExitStack() as ctx:
        pool = lambda name, bufs, **kw: ctx.enter_context(
            tc.tile_pool(name=name, bufs=bufs, **kw))
        cst = pool("cst", 1)
        qk = pool("qk", 2)
        vpool = pool("vpool", 2)
        xp = pool("xp", 4)
        wvp = pool("wvp", 3)
        sp = pool("sp", 3)        # pt tiles
        rp = pool("rp", 2)        # rope temps
        tpp = pool("tpp", 2)      # lora-down staging
        mp = pool("mp", 3)        # merge temps
        shp = pool("shp", 3)      # att/ship tiles
        wop = pool("wop", 3)      # woT stream
        atp = pool("atp", 4)      # attT stream
        osp = pool("osp", 2)      # out staging
        # PSUM: 4 + 2 + 2 banks.  All [128,512]-accumulator users share one
        # tag so attention blocks, projection sweeps and the wo phase can
        # overlap without exceeding the 8 banks.
        ps_acc = pool("ps_acc", 4, space="PSUM")
        ps_sc = pool("ps_sc", 2, space="PSUM")
        ps_small = pool("ps_small", 2, space="PSUM")

        def acc_tile():
            return ps_acc.tile([128, 512], f32, tag="acc", name="acct")

        # ---------------- constants / weights preload ----------------
        wq_sb = cst.tile([128, NI, QO], bfl, tag="wq")
        nc.sync.dma_start(wq_sb[:], wqT[:].rearrange("(a p) f -> p a f", p=128))
        wk_sb = cst.tile([128, NI, QO], bfl, tag="wk")
        nc.sync.dma_start(wk_sb[:], wkT[:].rearrange("(a p) f -> p a f", p=128))
        l1_sb = cst.tile([128, NI, 96], bfl, tag="l1")
        nc.sync.dma_start(l1_sb[:], l1a[:].rearrange("(a p) f -> p a f", p=128))
        l2a_sb = cst.tile([96, QO], bfl, tag="l2a")
        nc.sync.dma_start(l2a_sb[:], l2ad[:])
        admcol = cst.tile([32, 1], bfl, tag="admcol")
        nc.sync.dma_start(admcol[:], admd[:])
        l1o_sb = cst.tile([128, NI, R], bfl, tag="l1o")
        nc.sync.dma_start(l1o_sb[:], l1od[:].rearrange("(a p) f -> p a f", p=128))
        ad_sb = cst.tile([128, NI, B * 32], bfl, tag="ad")
        nc.sync.dma_start(ad_sb[:], adT[:].rearrange("(a p) f -> p a f", p=128))
        c2_sb = cst.tile([128, S], bfl, tag="c2")
        nc.sync.dma_start(c2_sb[:], c2d[:])
        s2_sb = cst.tile([128, S], bfl, tag="s2")
        nc.sync.dma_start(s2_sb[:], s2d[:])
        tg_sb = cst.tile([128, HL], f32, tag="tg")
        nc.sync.dma_start(tg_sb[:], tgd[:])
        if n_gen:
            em_sb = cst.tile([128, n_gen, 512], bfl, tag="em")
            nc.sync.dma_start(em_sb[:], embd[:].rearrange("g p f -> p g f"))
        ident = cst.tile([128, 128], bfl, tag="ident")
        make_identity(nc, ident[:])
        onecol = cst.tile([128, 1], bfl, tag="onecol")
        nc.vector.memset(onecol[:], 1.0)

        # ---------------- adapter k/v (base weights, no rope/lora) --------
        akn_ps = acc_tile()
        avn_ps = acc_tile()
        for i in range(NI):
            wvt = wvp.tile([128, QO], bfl, tag="wv")
            nc.sync.dma_start(wvt[:], wvT[i * 128:(i + 1) * 128, :])
            nc.tensor.matmul(akn_ps[:], ad_sb[:, i, :], wk_sb[:, i, :],
                             start=(i == 0), stop=(i == NI - 1))
            nc.tensor.matmul(avn_ps[:], ad_sb[:, i, :], wvt[:],
                             start=(i == 0), stop=(i == NI - 1))
        akn_sb = cst.tile([B * 32, QO], bfl, tag="akn")
        nc.scalar.copy(akn_sb[:], akn_ps[:])
        av_sb = cst.tile([32, B, HL, HD], bfl, tag="av")
        for b2 in range(B):
            nc.vector.tensor_copy(
                av_sb[:, b2].rearrange("p h d -> p (h d)"),
                avn_ps[b2 * 32:(b2 + 1) * 32, :])
        akT_sb = cst.tile([128, HL, B * 32], bfl, tag="akT")
        for hh in range(HL):
            trp = ps_small.tile([128, B * 32], bfl, tag="small", name="aktr")
            nc.tensor.transpose(trp[:], akn_sb[:, hh * HD:(hh + 1) * HD],
                                ident[:])
            nc.scalar.copy(akT_sb[:, hh, :], trp[:])

        # ---------------- per-batch: projection + attention ----------------
        tmpo_sb = cst.tile([R, B, 128], bfl, tag="tmpo")
        for b in range(B):
            qT_sb = qk.tile([128, HL, S], bfl, tag="qT")
            kT_sb = qk.tile([128, HL, S], bfl, tag="kT")
            v_sb = vpool.tile([128, NJ, QO], bfl, tag="v")

            for t2 in range(NT2):
                ts = t2 * 512
                tg0 = b * S + ts
                xtc = []
                for cchunk in range(4):
                    xt = xp.tile([128, 8, 512], bfl, tag="xt")
                    src = xT[cchunk * 1024:(cchunk + 1) * 1024, tg0:tg0 + 512]
                    nc.sync.dma_start(xt[:], src.rearrange("(a p) t -> p a t", p=128))
                    xtc.append(xt)

                def xtile(i):
                    return xtc[i // 8][:, i % 8, :]

                def rope_store(ps, dst, ts_):
                    cs = c2_sb[:, ts_:ts_ + 512]
                    sn = s2_sb[:, ts_:ts_ + 512]
                    qb = rp.tile([128, 512], bfl, tag="ropeq")
                    nc.scalar.copy(qb[:], ps[:])
                    sw = rp.tile([128, 512], bfl, tag="ropesw")
                    nc.vector.tensor_copy(sw[0:64, :], qb[64:128, :])
                    nc.vector.tensor_copy(sw[64:128, :], qb[0:64, :])
                    nc.vector.tensor_mul(sw[:], sw[:], sn)
                    nc.vector.tensor_mul(qb[:], qb[:], cs)
                    nc.vector.tensor_add(dst, qb[:], sw[:])

                # ---- q heads + lora down-projection (2 heads / sweep) ----
                tmp_ps = ps_small.tile([96, 512], f32, tag="small", name="tmp_ps")
                tmp_sb = tpp.tile([96, 512], bfl, tag="tmps")
                for half in range(2):
                    qps = [acc_tile() for _ in range(2)]
                    for i in range(NI):
                        for hx in range(2):
                            hh = half * 2 + hx
                            nc.tensor.matmul(qps[hx][:],
                                             wq_sb[:, i, hh * HD:(hh + 1) * HD],
                                             xtile(i), start=(i == 0), stop=False)
                        if half == 0:
                            nc.tensor.matmul(tmp_ps[:], l1_sb[:, i, :], xtile(i),
                                             start=(i == 0), stop=(i == NI - 1))
                    if half == 0:
                        nc.scalar.copy(tmp_sb[:], tmp_ps[:])
                    for hx in range(2):
                        hh = half * 2 + hx
                        nc.tensor.matmul(qps[hx][:],
                                         l2a_sb[0:R, hh * HD:(hh + 1) * HD],
                                         tmp_sb[0:R, :], start=False, stop=True)
                        rope_store(qps[hx], qT_sb[:, hh, ts:ts + 512], ts)
                # ---- k heads ----
                for half in range(2):
                    kps = [acc_tile() for _ in range(2)]
                    for i in range(NI):
                        for hx in range(2):
                            hh = half * 2 + hx
                            nc.tensor.matmul(kps[hx][:],
                                             wk_sb[:, i, hh * HD:(hh + 1) * HD],
                                             xtile(i), start=(i == 0), stop=False)
                    for hx in range(2):
                        hh = half * 2 + hx
                        nc.tensor.matmul(kps[hx][:],
                                         l2a_sb[32:32 + R, hh * HD:(hh + 1) * HD],
                                         tmp_sb[32:32 + R, :], start=False, stop=True)
                        rope_store(kps[hx], kT_sb[:, hh, ts:ts + 512], ts)
                # ---- v natural [token, feature] ----
                for half in range(2):
                    vps = [acc_tile() for _ in range(2)]
                    for i in range(NI):
                        wvt = wvp.tile([128, QO], bfl, tag="wv")
                        nc.sync.dma_start(wvt[:], wvT[i * 128:(i + 1) * 128, :])
                        for sx in range(2):
                            sub = half * 2 + sx
                            nc.tensor.matmul(vps[sx][:],
                                             xtile(i)[:, sub * 128:(sub + 1) * 128],
                                             wvt[:], start=(i == 0), stop=False)
                    for sx in range(2):
                        sub = half * 2 + sx
                        nc.tensor.matmul(vps[sx][:],
                                         tmp_sb[64:64 + R, sub * 128:(sub + 1) * 128],
                                         l2a_sb[64:64 + R, :], start=False, stop=True)
                        nc.scalar.copy(v_sb[:, t2 * 4 + sub, :], vps[sx][:])

            # ---- attention for batch b ----
            for hh in range(HL):
                for T in range(NT2):
                    out_data = acc_tile()
                    ad_data = acc_tile()
                    sums = ps_small.tile([128, 8], f32, tag="small", name="sums")
                    # PSUM start=True clears has_written for the WHOLE bank:
                    # exactly one start=True per psum tile (its first mm).
                    started = {"out": False, "ad": False, "sums": False}

                    def gmm(which, out_ap, lhsT, rhs, stop):
                        nc.tensor.matmul(out_ap, lhsT, rhs,
                                         start=not started[which], stop=stop)
                        started[which] = True

                    for j in range(NJ):
                        kind, gidx = cls[(T, j)]
                        if kind == "zero":
                            continue
                        sc = ps_sc.tile([128, 512], f32, tag="sc", name="sc")
                        nc.tensor.matmul(sc[:], kT_sb[:, hh, j * 128:(j + 1) * 128],
                                         qT_sb[:, hh, T * 512:(T + 1) * 512],
                                         start=True, stop=True)
                        pt = sp.tile([128, 512], bfl, tag="pt")
                        nc.scalar.activation(pt[:], sc[:], AF.Exp, scale=SCALE)
                        if kind == "gen":
                            nc.vector.tensor_mul(pt[:], pt[:], em_sb[:, gidx, :])
                        for s_ in range(4):
                            g = T * 4 + s_
                            if subzero[(g, j)]:
                                continue
                            stp = (j == live_j[g][-1])
                            gmm("out", out_data[:, s_ * 128:(s_ + 1) * 128],
                                pt[:, s_ * 128:(s_ + 1) * 128],
                                v_sb[:, j, hh * HD:(hh + 1) * HD], stp)
                            gmm("sums", sums[:, s_:s_ + 1],
                                pt[:, s_ * 128:(s_ + 1) * 128], onecol[:], stp)
                    # adapter cross-attention for (hh, T)
                    adsc = ps_sc.tile([32, 512], f32, tag="sc", name="adsc")
                    nc.tensor.matmul(adsc[:], akT_sb[:, hh, b * 32:(b + 1) * 32],
                                     qT_sb[:, hh, T * 512:(T + 1) * 512],
                                     start=True, stop=True)
                    pad = sp.tile([32, 512], bfl, tag="pad")
                    nc.scalar.activation(pad[:], adsc[:], AF.Exp, scale=SCALE)
                    for s_ in range(4):
                        gmm("ad", ad_data[:, s_ * 128:(s_ + 1) * 128],
                            pad[:, s_ * 128:(s_ + 1) * 128],
                            av_sb[:, b, hh, :], True)
                        gmm("sums", sums[:, 4 + s_:5 + s_],
                            pad[:, s_ * 128:(s_ + 1) * 128], admcol[:], True)
                    # normalize + merge + transpose + ship
                    rcpt = mp.tile([128, 16], f32, tag="rcp")
                    for s_ in range(4):
                        g = T * 4 + s_
                        att = shp.tile([128, 128], bfl, tag="att")
                        if not live_j[g]:
                            nc.vector.memset(att[:], 0.0)
                        else:
                            rc = rcpt[:, s_:s_ + 1]
                            rca = rcpt[:, 8 + s_:9 + s_]
                            nc.vector.reciprocal(rc, sums[:, s_:s_ + 1])
                            nc.vector.reciprocal(rca, sums[:, 4 + s_:5 + s_])
                            rca2 = rcpt[:, 12 + s_:13 + s_]
                            nc.vector.tensor_mul(rca2, rca, tg_sb[:, hh:hh + 1])
                            tmpm = mp.tile([128, 128], f32, tag="mtmp")
                            nc.vector.tensor_scalar_mul(
                                tmpm[:], ad_data[:, s_ * 128:(s_ + 1) * 128], rca2)
                            nc.vector.scalar_tensor_tensor(
                                out=att[:],
                                in0=out_data[:, s_ * 128:(s_ + 1) * 128],
                                scalar=rc, in1=tmpm[:],
                                op0=ALU.mult, op1=ALU.add)
                        trp = ps_small.tile([128, 128], bfl, tag="small", name="trp")
                        nc.tensor.transpose(trp[:], att[:], ident[:])
                        attt = shp.tile([128, 128], bfl, tag="attt")
                        nc.scalar.copy(attt[:], trp[:])
                        nc.scalar.dma_start(
                            a2a_in[b, g, hh * HD:(hh + 1) * HD, :], attt[:])
            nc.gpsimd.collective_compute(
                "AllToAll", mybir.AluOpType.bypass,
                ins=[a2a_in[b]], outs=[a2a_out[b]],
                replica_groups=[list(range(NCORES))],
            )
            # lora-o down-projection for this batch's token group
            top = ps_small.tile([R, 128], f32, tag="small", name="top")
            for a in range(NI):
                at4 = atp.tile([128, 128], bfl, tag="at4", name="attmpo")
                nc.gpsimd.dma_start(
                    at4[:], a2a_out[b, a // HL, (a % HL) * HD:(a % HL + 1) * HD, :])
                nc.tensor.matmul(top[:], l1o_sb[:, a, :], at4[:],
                                 start=(a == 0), stop=(a == NI - 1))
            nc.scalar.copy(tmpo_sb[:, b, :], top[:])

        # ---------------- wo projection (token-parallel) ----------------
        # attT af-tile a = (src core a//HL, local head a%HL); token group = core.
        for o in range(8):
            l2ot = wop.tile([R, 512], bfl, tag="l2o")
            nc.gpsimd.dma_start(l2ot[:], l2od[:, o * 512:(o + 1) * 512])
            wps = [acc_tile() for _ in range(B)]
            for a in range(NI):
                wot = wop.tile([128, 512], bfl, tag="wot")
                nc.gpsimd.dma_start(
                    wot[:], woT[a * 128:(a + 1) * 128, o * 512:(o + 1) * 512])
                at4 = atp.tile([128, B, 128], bfl, tag="at4", name="at4")
                nc.gpsimd.dma_start(
                    at4[:],
                    a2a_out[:, a // HL, (a % HL) * HD:(a % HL + 1) * HD, :]
                    .rearrange("b p t -> p b t"))
                for b2 in range(B):
                    nc.tensor.matmul(wps[b2][:], at4[:, b2, :], wot[:],
                                     start=(a == 0), stop=False)
            for b2 in range(B):
                nc.tensor.matmul(wps[b2][:], tmpo_sb[:, b2, :], l2ot[:],
                                 start=False, stop=True)
                ost = osp.tile([128, 512], f32, tag="ost")
                nc.scalar.copy(ost[:], wps[b2][:])
                nc.scalar.dma_start(out[b2, :, o * 512:(o + 1) * 512], ost[:])

    nc.compile()
    return nc


# --------------------------------------------------------------------------
# host side
# --------------------------------------------------------------------------
def _prep_inputs(inputs, meta):
    f = lambda k: np.asarray(inputs[k], np.float32)
    x = f("x").reshape(B * S, DIM)
    xT = np.ascontiguousarray(x.astype(bf16).T)                  # [DIM, B*S]
    ad_pad = np.zeros((B, 32, DIM), np.float32)
    ad_pad[:, :AL, :] = f("adapter")
    adT = np.ascontiguousarray(ad_pad.reshape(B * 32, DIM).astype(bf16).T)
    admcol = np.zeros((32, 1), np.float32)
    admcol[:AL] = 1.0
    admcol = admcol.astype(bf16)
    wq, wk, wv, wo = f("wq"), f("wk"), f("wv"), f("wo")
    l2q_, l2k_, l2v_ = f("lora_wq_l2"), f("lora_wk_l2"), f("lora_wv_l2")
    l1a_ = np.zeros((96, DIM), np.float32)
    l1a_[0:R] = f("lora_wq_l1")
    l1a_[32:32 + R] = f("lora_wk_l1")
    l1a_[64:64 + R] = f("lora_wv_l1")
    l1a = np.ascontiguousarray(l1a_.astype(bf16).T)              # [DIM, 96]
    woT = np.ascontiguousarray(wo.astype(bf16).T)                # [DIM, DIM]
    l1o = np.ascontiguousarray(f("lora_wo_l1").astype(bf16).T)   # [DIM, R]
    l2o = np.ascontiguousarray(f("lora_wo_l2").astype(bf16).T)   # [R, DIM]
    cos, sin = f("freqs_cos"), f("freqs_sin")                    # [S, HD/2]
    c2 = np.ascontiguousarray(np.concatenate([cos.T, cos.T], axis=0).astype(bf16))
    s2 = np.ascontiguousarray(np.concatenate([-sin.T, sin.T], axis=0).astype(bf16))
    gate = f("gate").reshape(H)
    perm = np.concatenate([np.arange(0, HD, 2), np.arange(1, HD, 2)])

    gen = meta["gen"]
    embl = np.stack(gen, axis=0) if gen else None

    in_maps = []
    for c in range(NCORES):
        rows_n = c * QO + np.arange(QO)                          # natural rows
        rows_p = np.concatenate(
            [(c * HL + hh) * HD + perm for hh in range(HL)])     # permuted rows
        tanhg = np.broadcast_to(
            np.tanh(gate[c * HL:(c + 1) * HL]).astype(np.float32), (128, HL))
        l2all_ = np.zeros((96, QO), np.float32)
        l2all_[0:R] = l2q_[rows_p].T
        l2all_[32:32 + R] = l2k_[rows_p].T
        l2all_[64:64 + R] = l2v_[rows_n].T
        l2all = np.ascontiguousarray(l2all_.astype(bf16))
        m = {
            "xT": xT,
            "wqT": np.ascontiguousarray(wq[rows_p].astype(bf16).T),
            "wkT": np.ascontiguousarray(wk[rows_p].astype(bf16).T),
            "wvT": np.ascontiguousarray(wv[rows_n].astype(bf16).T),
            "woT": woT,
            "l1a": l1a,
            "l2all": l2all,
            "l1o": l1o,
            "l2o": l2o,
            "adT": adT,
            "c2": c2,
            "s2": s2,
            "tanhg": np.ascontiguousarray(tanhg),
            "admcol": admcol,
        }
        if embl is not None:
            m["embl"] = embl
        in_maps.append(m)
    return in_maps


def _get_program(mask):
    key, meta = _classify_mask(mask)
    if key not in _CACHE:
        _CACHE[key] = (_build(meta), meta)
    return _CACHE[key]


def _run(inputs, trace=False, trace_kwargs=None):
    from concourse.bass_utils import run_bass_kernel_spmd

    nc, meta = _get_program(np.asarray(inputs["mask"], np.float32))
    in_maps = _prep_inputs(inputs, meta)
    res = run_bass_kernel_spmd(nc, in_maps, list(range(NCORES)),
                               trace=trace, **(trace_kwargs or {}))
    final = np.empty((B, S, DIM), np.float32)
    for c in range(NCORES):
        oc = res.results[c]["out"]                               # [B, 128, DIM]
        final[:, c * 128:(c + 1) * 128, :] = oc
    return final, res


def kernel(**inputs) -> np.ndarray:
    out, _ = _run(inputs)
    return out
